# revision 1
# baseline (speedup 1.0000x reference)
"""Trainium2 Bass kernel for the CustomGNN message-passing model.

Strategy (dst-sharded, 8 cores, bf16 edge phase):
  - Nodes padded to 50176, split into 8 shards of 6272 (49 windows of 128).
  - Per layer:
      P1a: own-shard D/A projections (fp32) from the SBUF-resident x shard
           (PE transpose + matmul); Dx -> SBUF bf16, Ax -> SBUF fp32.
      P1b: E/B projections for ALL nodes (bf16 matmuls from the AllGathered
           feature-major x) -> HBM gather table [50176, 256] bf16, viewed as
           pair rows [25088, 512] for gathering.
      P2:  per 128-dst-node window: per 128-edge tile, one indirect DMA
           gathers a 1KB pair row per lane (pair idx = src>>1); a per-lane
           parity select extracts [Ex|Bx]; sigma = sigmoid(Dx_bcast + Ex)
           where Dx_bcast = indT @ Dx_window on the PE; contrib = sigma*Bx;
           a one-hot slot-indicator matmul scatter-adds [contrib | sigma]
           into the PSUM window accumulator -> num/den.
      P3:  h = Ax + num/(den+eps) (fp32, SBUF); BN stats via mask-matmul +
           tiny AllReduce; BN apply + ReLU + residual updates the x shard;
           x is PE-transposed and AllGathered (bf16) for the next layer.
  - Final: segment mean-pool via batch-indicator matmul + AllReduce, then the
    MLP head computed redundantly on every core.
"""

import sys

if "/opt/trn_rl_repo" not in sys.path:
    sys.path.insert(0, "/opt/trn_rl_repo")

import numpy as np
import ml_dtypes

from concourse import bacc, bass, mybir, tile
from concourse.bass_utils import run_bass_kernel_spmd

F32 = mybir.dt.float32
BF16 = mybir.dt.bfloat16
I32 = mybir.dt.int32

P = 128
D = 128
L = 3
G = 64
DIM_OUT = 10
NCORES = 8
N_REAL = 50000
SH = 6272               # shard size (nodes per core), 49 windows of 128
WPC = SH // P           # windows per core = 49
NPAD = SH * NCORES      # 50176
EPS_BN = 1e-5
EPS_AGG = 1e-6

_cache = {}


def _pack_edges(src, dst):
    """Sort edges by dst, pack into per-core windows of 128 dst nodes.

    Per-window tile count T[w] is the max over cores at window position w.
    Returns (pk, sc, indt, tws):
      pk   [NCORES, 128, sumT] int32  — pair index (src>>1) per lane
      sc   [NCORES, 128, 3*sumT] f32  — per tile j: col 3j = 1-(src&1),
                                        3j+1 = src&1, 3j+2 = slot (or -1)
      indt [NCORES, 128, sumT*128] bf16 — transposed slot indicator per tile
      tws  list[int] per-window tile counts
    """
    order = np.argsort(dst, kind="stable")
    src_s = src[order].astype(np.int64)
    dst_s = dst[order].astype(np.int64)
    win = dst_s // P                      # global window id = 49*c + w
    nw = NCORES * WPC
    counts = np.bincount(win, minlength=nw)
    tpw = np.maximum((counts + P - 1) // P, 1).reshape(NCORES, WPC)
    tws = tpw.max(axis=0)                 # [WPC] max over cores
    sumt = int(tws.sum())
    toff = np.zeros(WPC + 1, np.int64)
    np.cumsum(tws, out=toff[1:])

    starts = np.zeros(nw + 1, np.int64)
    np.cumsum(counts, out=starts[1:])
    pos = np.arange(len(dst_s)) - starts[win]   # rank within window

    srcA = np.zeros((NCORES, sumt * P), np.int64)
    slotA = np.full((NCORES, sumt * P), -1.0, np.float32)
    core = win // WPC
    wloc = win % WPC
    flat = toff[wloc] * P + pos
    srcA[core, flat] = src_s
    slotA[core, flat] = (dst_s % P).astype(np.float32)

    # [core, sumt, P] -> lane p of tile j is edge j*128+p
    srcT = srcA.reshape(NCORES, sumt, P).transpose(0, 2, 1)   # [c, p, j]
    slotT = slotA.reshape(NCORES, sumt, P).transpose(0, 2, 1)

    pk = (srcT >> 1).astype(np.int32)
    par = (srcT & 1).astype(np.float32)
    sc = np.empty((NCORES, P, 3 * sumt), np.float32)
    sc[:, :, 0::3] = 1.0 - par
    sc[:, :, 1::3] = par
    sc[:, :, 2::3] = slotT

    # indT[m, e] for tile j: (slot[e] == m); padding (slot=-1) -> zero col
    sl = slotA.reshape(NCORES, sumt, P)                        # [c, j, e]
    m = np.arange(P, dtype=np.float32)
    indt = (sl[:, :, None, :] == m[None, None, :, None])       # [c, j, m, e]
    indt = indt.astype(ml_dtypes.bfloat16).reshape(
        NCORES, sumt, P, P).transpose(0, 2, 1, 3).reshape(NCORES, P, sumt * P)
    return pk, np.ascontiguousarray(sc), np.ascontiguousarray(indt), \
        [int(t) for t in tws]


def _build(tws, dbg=False):
    """Build + compile the SPMD Bass program for per-window tile counts."""
    sumt = sum(tws)
    toff = [0]
    for t in tws:
        toff.append(toff[-1] + t)

    nc = bacc.Bacc("TRN2", target_bir_lowering=False, debug=False,
                   num_devices=NCORES)

    # ---- I/O -------------------------------------------------------------
    xf0 = nc.dram_tensor("xf0", [NCORES * P, SH], BF16, kind="ExternalInput")
    xn0 = nc.dram_tensor("xn0", [SH, D], F32, kind="ExternalInput")
    wtsf = nc.dram_tensor("wtsf", [L * 2 * D, D], F32, kind="ExternalInput")
    wtsb = nc.dram_tensor("wtsb", [L * 2 * D, D], BF16, kind="ExternalInput")
    bias_eb = nc.dram_tensor("bias_eb", [P, L * 2 * D], F32, kind="ExternalInput")
    bias_da = nc.dram_tensor("bias_da", [P, L * 2 * D], F32, kind="ExternalInput")
    gb = nc.dram_tensor("gb", [1, L * 2 * D], F32, kind="ExternalInput")
    pk = nc.dram_tensor("pk", [P, sumt], I32, kind="ExternalInput")
    sct = nc.dram_tensor("sct", [P, 3 * sumt], F32, kind="ExternalInput")
    indt = nc.dram_tensor("indt", [P, sumt * P], BF16, kind="ExternalInput")
    maskT = nc.dram_tensor("maskT", [P, WPC], F32, kind="ExternalInput")
    bslotT = nc.dram_tensor("bslotT", [P, WPC], F32, kind="ExternalInput")
    iota128 = nc.dram_tensor("iota128", [P, P], BF16, kind="ExternalInput")
    iota64 = nc.dram_tensor("iota64", [P, G], F32, kind="ExternalInput")
    ident = nc.dram_tensor("ident", [P, P], F32, kind="ExternalInput")
    ones1 = nc.dram_tensor("ones1", [1, P], F32, kind="ExternalInput")
    onescol = nc.dram_tensor("onescol", [P, 1], F32, kind="ExternalInput")
    rcnt = nc.dram_tensor("rcnt", [G, 1], F32, kind="ExternalInput")
    pw1 = nc.dram_tensor("pw1", [D, D], F32, kind="ExternalInput")
    pb1b = nc.dram_tensor("pb1b", [G, D], F32, kind="ExternalInput")
    pw2 = nc.dram_tensor("pw2", [D, DIM_OUT], F32, kind="ExternalInput")
    pb2b = nc.dram_tensor("pb2b", [G, DIM_OUT], F32, kind="ExternalInput")
    out_t = nc.dram_tensor("out", [G, DIM_OUT], F32, kind="ExternalOutput")

    # ---- internal DRAM ---------------------------------------------------
    eb_tabs = [nc.dram_tensor(f"eb_tab{l}", [NPAD, 2 * D], BF16)
               for l in range(L)]
    agin = [nc.dram_tensor(f"agin{l}", [P, SH], BF16) for l in range(L - 1)]
    agout = [
        nc.dram_tensor(f"agout{l}", [NCORES * P, SH], BF16, addr_space="Shared")
        for l in range(L - 1)
    ]
    stin = [nc.dram_tensor(f"stin{l}", [1, 2 * D], F32) for l in range(L)]
    stout = [
        nc.dram_tensor(f"stout{l}", [1, 2 * D], F32, addr_space="Shared")
        for l in range(L)
    ]
    plin = nc.dram_tensor("plin", [G, D], F32)
    plout = nc.dram_tensor("plout", [G, D], F32, addr_space="Shared")
    scshd = [nc.dram_tensor(f"scshd{l}", [1, 2 * D], F32) for l in range(L)]
    if dbg:
        t0 = tws[0]
        hdbg = nc.dram_tensor("hdbg", [P, WPC * P], F32, kind="ExternalOutput")
        seldbg = nc.dram_tensor("seldbg", [P, t0 * 2 * D], BF16,
                                kind="ExternalOutput")
        inddbg = nc.dram_tensor("inddbg", [P, t0 * P], BF16,
                                kind="ExternalOutput")
        rhsdbg = nc.dram_tensor("rhsdbg", [P, t0 * 2 * D], BF16,
                                kind="ExternalOutput")
        windbg = nc.dram_tensor("windbg", [P, 2 * D], F32,
                                kind="ExternalOutput")
        dxdbg = nc.dram_tensor("dxdbg", [P, WPC * P], BF16,
                               kind="ExternalOutput")
        stdbg = nc.dram_tensor("stdbg", [2, 2 * D], F32, kind="ExternalOutput")
        xdbgs = [nc.dram_tensor(f"xdbg{l}", [P, WPC * P], F32,
                                kind="ExternalOutput") for l in range(L)]
        hdbgs = [nc.dram_tensor(f"hdbg{l}", [P, WPC * P], F32,
                                kind="ExternalOutput") for l in range(1, L)]
        pldbg = nc.dram_tensor("pldbg", [2, G, D + 1], F32,
                               kind="ExternalOutput")
        pooldbg = nc.dram_tensor("pooldbg", [G, D], F32, kind="ExternalOutput")
        h1dbg = nc.dram_tensor("h1dbg", [G, D], F32, kind="ExternalOutput")
        agdbg = nc.dram_tensor("agdbg", [NCORES * P, SH], BF16,
                               kind="ExternalOutput")

    RG = [list(range(NCORES))]
    ADD = mybir.AluOpType.add
    MUL = mybir.AluOpType.mult
    SUB = mybir.AluOpType.subtract
    EQ = mybir.AluOpType.is_equal
    AF = mybir.ActivationFunctionType

    with tile.TileContext(nc) as tc:
        ctxs = []

        def pool(**kw):
            p_ = tc.tile_pool(**kw)
            ctxs.append(p_)
            return p_.__enter__()

        persist = pool(name="persist", bufs=1)
        p_xf = pool(name="p_xf", bufs=2)
        p_stage = pool(name="p_stage", bufs=2)
        p_w = pool(name="p_w", bufs=2)        # per-window buffers
        p_t = pool(name="p_t", bufs=3)        # per-tile small buffers
        p_small = pool(name="p_small", bufs=2)
        p_tiny = pool(name="p_tiny", bufs=1)
        ps_t = pool(name="ps_t", bufs=2, space="PSUM")
        ps_proj = pool(name="ps_proj", bufs=3, space="PSUM")
        ps_win = pool(name="ps_win", bufs=2, space="PSUM")
        ps_acc = pool(name="ps_acc", bufs=1, space="PSUM")

        # ---- persistent SBUF --------------------------------------------
        xN = persist.tile([P, WPC * P], F32)       # x shard, node-major
        Ax = persist.tile([P, WPC * P], F32)
        Dxb = persist.tile([P, WPC * P], BF16)
        Hs = persist.tile([P, WPC * P], F32)
        pksb = persist.tile([P, sumt], I32)
        scsb = persist.tile([P, 3 * sumt], F32)
        Wf = persist.tile([P, L * 2 * D], F32)
        Wb = persist.tile([P, L * 2 * D], BF16)
        bEB = persist.tile([P, L * 2 * D], F32)
        bDA = persist.tile([P, L * 2 * D], F32)
        gbsb = persist.tile([1, L * 2 * D], F32)
        io128 = persist.tile([P, P], BF16)
        io64 = persist.tile([P, G], F32)
        idsb = persist.tile([P, P], F32)
        onescolsb = persist.tile([P, 1], F32)
        masksb = persist.tile([P, WPC], F32)
        bslotsb = persist.tile([P, WPC], F32)
        rcntsb = persist.tile([G, 1], F32)
        pw1sb = persist.tile([P, D], F32)
        pb1sb = persist.tile([G, D], F32)
        pw2sb = persist.tile([P, DIM_OUT], F32)
        pb2sb = persist.tile([G, DIM_OUT], F32)

        nc.sync.dma_start(out=xN[:].rearrange("p (w d) -> p w d", d=D),
                          in_=xn0[:].rearrange("(w p) d -> p w d", p=P))
        nc.sync.dma_start(out=pksb[:], in_=pk[:, :])
        nc.sync.dma_start(out=scsb[:], in_=sct[:, :])
        nc.sync.dma_start(out=Wf[:].rearrange("p (k d) -> p k d", d=D),
                          in_=wtsf[:].rearrange("(k p) d -> p k d", p=P))
        nc.sync.dma_start(out=Wb[:].rearrange("p (k d) -> p k d", d=D),
                          in_=wtsb[:].rearrange("(k p) d -> p k d", p=P))
        nc.sync.dma_start(out=bEB[:], in_=bias_eb[:, :])
        nc.sync.dma_start(out=bDA[:], in_=bias_da[:, :])
        nc.sync.dma_start(out=gbsb[:], in_=gb[:, :])
        nc.sync.dma_start(out=io128[:], in_=iota128[:, :])
        nc.sync.dma_start(out=io64[:], in_=iota64[:, :])
        nc.sync.dma_start(out=idsb[:], in_=ident[:, :])
        nc.sync.dma_start(out=onescolsb[:], in_=onescol[:, :])
        nc.sync.dma_start(out=masksb[:], in_=maskT[:, :])
        nc.sync.dma_start(out=bslotsb[:], in_=bslotT[:, :])
        nc.sync.dma_start(out=rcntsb[:], in_=rcnt[:, :])
        nc.sync.dma_start(out=pw1sb[:], in_=pw1[:, :])
        nc.sync.dma_start(out=pb1sb[:], in_=pb1b[:, :])
        nc.sync.dma_start(out=pw2sb[:], in_=pw2[:, :])
        nc.sync.dma_start(out=pb2sb[:], in_=pb2b[:, :])

        def Wfs(l, k):   # k: 0=D, 1=A
            return Wf[:, (l * 2 + k) * D:(l * 2 + k + 1) * D]

        def Wbs(l, k):   # k: 0=E, 1=B
            return Wb[:, (l * 2 + k) * D:(l * 2 + k + 1) * D]

        STG = 4

        for l in range(L):
            xsrc = xf0 if l == 0 else agout[l - 1]
            eb_tab = eb_tabs[l]
            eb_pairs = eb_tab[:].rearrange("(q two) d -> q (two d)", two=2)

            # ---- P1a: own-shard D/A (fp32) -----------------------------
            for w in range(WPC):
                pt = ps_t.tile([P, P], F32, tag="pt")
                nc.tensor.transpose(
                    out=pt[:], in_=xN[:, w * P:(w + 1) * P], identity=idsb[:])
                xT = p_small.tile([P, P], F32, tag="xT")
                nc.scalar.copy(out=xT[:], in_=pt[:])
                pda = ps_proj.tile([P, 2 * D], F32, tag="proj")
                nc.tensor.matmul(out=pda[:, 0:D], lhsT=xT[:], rhs=Wfs(l, 0),
                                 start=True, stop=True)
                nc.tensor.matmul(out=pda[:, D:2 * D], lhsT=xT[:], rhs=Wfs(l, 1),
                                 start=True, stop=True)
                nc.vector.tensor_tensor(
                    out=Dxb[:, w * P:(w + 1) * P], in0=pda[:, 0:D],
                    in1=bDA[:, l * 2 * D:l * 2 * D + D], op=ADD)
                nc.vector.tensor_tensor(
                    out=Ax[:, w * P:(w + 1) * P], in0=pda[:, D:2 * D],
                    in1=bDA[:, l * 2 * D + D:(l + 1) * 2 * D], op=ADD)

            # ---- P1b: full-table E/B (bf16) ----------------------------
            CH = 8
            for s in range(NCORES):
                for c0 in range(0, WPC, CH):
                    ntl = min(CH, WPC - c0)
                    xf = p_xf.tile([P, CH * P], BF16, tag="xf")
                    nc.sync.dma_start(
                        out=xf[:, 0:ntl * P],
                        in_=xsrc[s * P:(s + 1) * P, c0 * P:(c0 + ntl) * P])
                    for t0 in range(0, ntl, STG):
                        nst = min(STG, ntl - t0)
                        ebs = p_stage.tile([P, STG * 2 * D], BF16, tag="ebs")
                        for q in range(nst):
                            t = t0 + q
                            peb = ps_proj.tile([P, 2 * D], F32, tag="proj")
                            nc.tensor.matmul(
                                out=peb[:, 0:D], lhsT=xf[:, t * P:(t + 1) * P],
                                rhs=Wbs(l, 0), start=True, stop=True)
                            nc.tensor.matmul(
                                out=peb[:, D:2 * D],
                                lhsT=xf[:, t * P:(t + 1) * P],
                                rhs=Wbs(l, 1), start=True, stop=True)
                            nc.vector.tensor_tensor(
                                out=ebs[:, q * 2 * D:(q + 1) * 2 * D],
                                in0=peb[:],
                                in1=bEB[:, l * 2 * D:(l + 1) * 2 * D], op=ADD)
                        base = s * SH + (c0 + t0) * P
                        dst_rows = eb_tab[base:base + nst * P, :].rearrange(
                            "(q p) d -> p q d", p=P)
                        nc.sync.dma_start(
                            out=dst_rows,
                            in_=ebs[:, 0:nst * 2 * D].rearrange(
                                "p (q d) -> p q d", d=2 * D))

            # ---- P2: gather + scatter per window -----------------------
            acc1 = p_small.tile([P, D], F32, tag="acc1")
            acc2 = p_small.tile([P, D], F32, tag="acc2")
            nc.gpsimd.memset(acc1[:], 0.0)
            nc.gpsimd.memset(acc2[:], 0.0)
            tmax = max(tws)
            for w in range(WPC):
                tw = tws[w]
                off = toff[w]
                tI = p_w.tile([P, tmax * P], BF16, tag="tI")
                nc.sync.dma_start(out=tI[:, 0:tw * P],
                                  in_=indt[:, off * P:(off + tw) * P])
                selw = p_w.tile([P, tmax * 2 * D], BF16, tag="selw")
                indw = p_w.tile([P, tmax * P], BF16, tag="indw")
                rhsw = p_w.tile([P, tmax * 2 * D], BF16, tag="rhsw")
                for j in range(tw):
                    jj = off + j
                    g = p_t.tile([P, 2 * 2 * D], BF16, tag="g")
                    nc.gpsimd.indirect_dma_start(
                        out=g[:], out_offset=None, in_=eb_pairs,
                        in_offset=bass.IndirectOffsetOnAxis(
                            ap=pksb[:, jj:jj + 1], axis=0))
                    sel = selw[:, j * 2 * D:(j + 1) * 2 * D]
                    nc.vector.tensor_scalar(
                        out=sel, in0=g[:, 0:2 * D],
                        scalar1=scsb[:, 3 * jj:3 * jj + 1], scalar2=None,
                        op0=MUL)
                    nc.vector.scalar_tensor_tensor(
                        out=sel, in0=g[:, 2 * D:4 * D],
                        scalar=scsb[:, 3 * jj + 1:3 * jj + 2], in1=sel,
                        op0=MUL, op1=ADD)
                    nc.vector.tensor_scalar(
                        out=indw[:, j * P:(j + 1) * P], in0=io128[:],
                        scalar1=scsb[:, 3 * jj + 2:3 * jj + 3], scalar2=None,
                        op0=EQ)
                    pdx = ps_t.tile([P, P], F32, tag="pt")
                    nc.tensor.matmul(
                        out=pdx[:], lhsT=tI[:, j * P:(j + 1) * P],
                        rhs=Dxb[:, w * P:(w + 1) * P], start=True, stop=True)
                    nc.vector.tensor_tensor(
                        out=sel[:, 0:D], in0=pdx[:], in1=sel[:, 0:D], op=ADD)
                # sigma for the whole window
                nc.scalar.activation(
                    out=rhsw[:].rearrange("p (t d) -> p t d",
                                          d=2 * D)[:, 0:tw, D:2 * D],
                    in_=selw[:].rearrange("p (t d) -> p t d",
                                          d=2 * D)[:, 0:tw, 0:D],
                    func=AF.Sigmoid)
                # contrib = sigma * Bx
                nc.vector.tensor_tensor(
                    out=rhsw[:].rearrange("p (t d) -> p t d",
                                          d=2 * D)[:, 0:tw, 0:D],
                    in0=rhsw[:].rearrange("p (t d) -> p t d",
                                          d=2 * D)[:, 0:tw, D:2 * D],
                    in1=selw[:].rearrange("p (t d) -> p t d",
                                          d=2 * D)[:, 0:tw, D:2 * D],
                    op=MUL)
                pwin = ps_win.tile([P, 2 * D], F32, tag="win")
                for j in range(tw):
                    nc.tensor.matmul(
                        out=pwin[:], lhsT=indw[:, j * P:(j + 1) * P],
                        rhs=rhsw[:, j * 2 * D:(j + 1) * 2 * D],
                        start=(j == 0), stop=(j == tw - 1))
                if dbg and l == 0 and w == 0:
                    nc.sync.dma_start(out=seldbg[:, :],
                                      in_=selw[:, 0:tw * 2 * D])
                    nc.sync.dma_start(out=inddbg[:, :], in_=indw[:, 0:tw * P])
                    nc.sync.dma_start(out=rhsdbg[:, :],
                                      in_=rhsw[:, 0:tw * 2 * D])
                    wc = p_t.tile([P, 2 * D], F32, tag="wc")
                    nc.vector.tensor_copy(out=wc[:], in_=pwin[:])
                    nc.sync.dma_start(out=windbg[:, :], in_=wc[:])
                den = p_t.tile([P, D], F32, tag="den")
                nc.vector.tensor_scalar_add(out=den[:], in0=pwin[:, D:2 * D],
                                            scalar1=EPS_AGG)
                nc.vector.reciprocal(out=den[:], in_=den[:])
                hwin = Hs[:, w * P:(w + 1) * P]
                nc.vector.tensor_tensor(out=hwin, in0=pwin[:, 0:D],
                                        in1=den[:], op=MUL)
                nc.vector.tensor_tensor(out=hwin, in0=hwin,
                                        in1=Ax[:, w * P:(w + 1) * P], op=ADD)
                hsq = p_t.tile([P, D], F32, tag="hsq")
                nc.scalar.activation(out=hsq[:], in_=hwin, func=AF.Square)
                nc.vector.scalar_tensor_tensor(
                    out=acc1[:], in0=hwin, scalar=masksb[:, w:w + 1],
                    in1=acc1[:], op0=MUL, op1=ADD)
                nc.vector.scalar_tensor_tensor(
                    out=acc2[:], in0=hsq[:], scalar=masksb[:, w:w + 1],
                    in1=acc2[:], op0=MUL, op1=ADD)

            if dbg and l == 0:
                nc.sync.dma_start(out=hdbg[:, :], in_=Hs[:])
                nc.sync.dma_start(out=dxdbg[:, :], in_=Dxb[:])
            if dbg and l > 0:
                nc.sync.dma_start(out=hdbgs[l - 1][:, :], in_=Hs[:])

            # ---- P3: BN stats allreduce + apply + update --------------
            stp = p_tiny.tile([1, 2 * D], F32, tag="stp")
            nc.gpsimd.tensor_reduce(out=stp[0:1, 0:D], in_=acc1[:],
                                    axis=mybir.AxisListType.C, op=ADD)
            nc.gpsimd.tensor_reduce(out=stp[0:1, D:2 * D], in_=acc2[:],
                                    axis=mybir.AxisListType.C, op=ADD)
            nc.sync.dma_start(out=stin[l][:, :], in_=stp[:])
            nc.gpsimd.collective_compute(
                "AllReduce", ADD, replica_groups=RG,
                ins=[stin[l][:, :]], outs=[stout[l][:, :]])
            ssum = p_tiny.tile([1, 2 * D], F32, tag="ssum")
            nc.sync.dma_start(out=ssum[:], in_=stout[l][:, :])

            scsh = p_tiny.tile([1, 2 * D], F32, tag="scsh")
            mu = p_tiny.tile([1, D], F32, tag="mu")
            tvar = p_tiny.tile([1, D], F32, tag="tvar")
            nc.vector.tensor_scalar_mul(out=mu[:], in0=ssum[0:1, 0:D],
                                        scalar1=1.0 / N_REAL)
            nc.vector.tensor_scalar_mul(out=tvar[:], in0=ssum[0:1, D:2 * D],
                                        scalar1=1.0 / N_REAL)
            musq = p_tiny.tile([1, D], F32, tag="musq")
            nc.vector.tensor_tensor(out=musq[:], in0=mu[:], in1=mu[:], op=MUL)
            nc.vector.tensor_tensor(out=tvar[:], in0=tvar[:], in1=musq[:],
                                    op=SUB)
            nc.vector.tensor_scalar_add(out=tvar[:], in0=tvar[:],
                                        scalar1=EPS_BN)
            nc.scalar.activation(out=tvar[:], in_=tvar[:], func=AF.Sqrt)
            nc.vector.reciprocal(out=tvar[:], in_=tvar[:])
            nc.vector.tensor_tensor(
                out=scsh[0:1, 0:D], in0=tvar[:],
                in1=gbsb[0:1, l * 2 * D:l * 2 * D + D], op=MUL)
            nc.vector.tensor_tensor(out=musq[:], in0=mu[:],
                                    in1=scsh[0:1, 0:D], op=MUL)
            nc.vector.tensor_tensor(
                out=scsh[0:1, D:2 * D],
                in0=gbsb[0:1, l * 2 * D + D:(l + 1) * 2 * D], in1=musq[:],
                op=SUB)
            scb = p_small.tile([P, 2 * D], F32, tag="scb")
            nc.sync.dma_start(out=scshd[l][:, :], in_=scsh[:])
            nc.sync.dma_start(
                out=scb[:],
                in_=scshd[l][0:1, :].to_broadcast([P, 2 * D]))

            last = l == L - 1
            if last:
                ppool = ps_win.tile([P, 2 * D], F32, tag="win")
            for w0 in range(0, WPC, STG):
                nw_ = min(STG, WPC - w0)
                if not last:
                    ags = p_stage.tile([P, STG * P], BF16, tag="ags")
                for q in range(nw_):
                    w = w0 + q
                    hv = p_t.tile([P, D], F32, tag="hv")
                    nc.vector.tensor_tensor(out=hv[:],
                                            in0=Hs[:, w * P:(w + 1) * P],
                                            in1=scb[:, 0:D], op=MUL)
                    nc.vector.tensor_tensor(out=hv[:], in0=hv[:],
                                            in1=scb[:, D:2 * D], op=ADD)
                    nc.scalar.activation(out=hv[:], in_=hv[:], func=AF.Relu)
                    nc.vector.tensor_tensor(
                        out=xN[:, w * P:(w + 1) * P],
                        in0=xN[:, w * P:(w + 1) * P], in1=hv[:], op=ADD)
                    if not last:
                        ptx = ps_t.tile([P, P], F32, tag="pt")
                        nc.tensor.transpose(out=ptx[:],
                                            in_=xN[:, w * P:(w + 1) * P],
                                            identity=idsb[:])
                        nc.scalar.copy(out=ags[:, q * P:(q + 1) * P],
                                       in_=ptx[:])
                    else:
                        ind64 = p_t.tile([P, G], F32, tag="ind64")
                        nc.vector.tensor_scalar(
                            out=ind64[:], in0=io64[:],
                            scalar1=bslotsb[:, w:w + 1], scalar2=None, op0=EQ)
                        nc.tensor.matmul(
                            out=ppool[0:G, 0:D], lhsT=ind64[:],
                            rhs=xN[:, w * P:(w + 1) * P],
                            start=(w == 0), stop=(w == WPC - 1))
                if not last:
                    nc.sync.dma_start(
                        out=agin[l][:, w0 * P:(w0 + nw_) * P],
                        in_=ags[:, 0:nw_ * P])
            if not last:
                nc.gpsimd.collective_compute(
                    "AllGather", mybir.AluOpType.bypass, replica_groups=RG,
                    ins=[agin[l][:, :]], outs=[agout[l][:, :]])
            if dbg and l == 0:
                nc.sync.dma_start(out=stdbg[0:1, :], in_=stin[0][:, :])
                nc.sync.dma_start(out=stdbg[1:2, :], in_=stout[0][:, :])
                nc.sync.dma_start(out=agdbg[:, :], in_=agout[0][:, :])
            if dbg:
                nc.sync.dma_start(out=xdbgs[l][:, :], in_=xN[:])

        # ---- readout: pooled mean + MLP ---------------------------------
        pls = p_small.tile([G, D], F32, tag="pls")
        nc.vector.tensor_copy(out=pls[:], in_=ppool[0:G, 0:D])
        nc.sync.dma_start(out=plin[:, :], in_=pls[:])
        if dbg:
            nc.sync.dma_start(out=pldbg[0, :, 0:D], in_=pls[:])
        nc.gpsimd.collective_compute(
            "AllReduce", ADD, replica_groups=RG,
            ins=[plin[:, :]], outs=[plout[:, :]])
        pl2 = p_small.tile([G, D], F32, tag="pl2")
        nc.sync.dma_start(out=pl2[:], in_=plout[:, :])
        pooled = p_small.tile([G, D], F32, tag="pooled")
        nc.vector.tensor_scalar(out=pooled[:], in0=pl2[:, 0:D],
                                scalar1=rcntsb[:, 0:1], scalar2=None, op0=MUL)
        if dbg:
            nc.sync.dma_start(out=pldbg[1, :, 0:D], in_=pl2[:])
            nc.sync.dma_start(out=pooldbg[:, :], in_=pooled[:])
        ptp = ps_t.tile([P, G], F32, tag="pt")
        nc.tensor.transpose(out=ptp[:], in_=pooled[:], identity=idsb[0:G, 0:G])
        pooledT = p_small.tile([P, G], F32, tag="pooledT")
        nc.scalar.copy(out=pooledT[:], in_=ptp[:])
        ph1 = ps_proj.tile([G, D], F32, tag="proj")
        nc.tensor.matmul(out=ph1[:], lhsT=pooledT[:], rhs=pw1sb[:],
                         start=True, stop=True)
        h1 = p_small.tile([G, D], F32, tag="h1")
        nc.vector.tensor_tensor(out=h1[:], in0=ph1[:], in1=pb1sb[:], op=ADD)
        nc.scalar.activation(out=h1[:], in_=h1[:], func=AF.Relu)
        if dbg:
            nc.sync.dma_start(out=h1dbg[:, :], in_=h1[:])
        pth = ps_t.tile([P, G], F32, tag="pt")
        nc.tensor.transpose(out=pth[:], in_=h1[:], identity=idsb[0:G, 0:G])
        h1T = p_small.tile([P, G], F32, tag="h1T")
        nc.scalar.copy(out=h1T[:], in_=pth[:])
        pout = ps_proj.tile([G, DIM_OUT], F32, tag="proj")
        nc.tensor.matmul(out=pout[:], lhsT=h1T[:], rhs=pw2sb[:],
                         start=True, stop=True)
        osb = p_small.tile([G, DIM_OUT], F32, tag="osb")
        nc.vector.tensor_tensor(out=osb[:], in0=pout[:], in1=pb2sb[:], op=ADD)
        nc.sync.dma_start(out=out_t[:, :], in_=osb[:])

        for p_ in reversed(ctxs):
            p_.__exit__(None, None, None)

    nc.compile()
    return nc


def _prepare_inputs(x, edge_index, batch_vec, lin_w, lin_b, bn_gamma, bn_beta,
                    post_w1, post_b1, post_w2, post_b2):
    x = np.asarray(x, np.float32)
    ei = np.asarray(edge_index)
    bv = np.asarray(batch_vec).astype(np.int64)
    lin_w = np.asarray(lin_w, np.float32)
    lin_b = np.asarray(lin_b, np.float32)

    pk, sc, indt, tws = _pack_edges(ei[0].astype(np.int64),
                                    ei[1].astype(np.int64))

    x0 = np.zeros((NPAD, D), np.float32)
    x0[:N_REAL] = x
    xf0 = np.ascontiguousarray(
        x0.reshape(NCORES, SH, D).transpose(0, 2, 1).reshape(
            NCORES * P, SH)).astype(ml_dtypes.bfloat16)

    # wtsf rows: (l*2+0)=D (k=2), (l*2+1)=A (k=0); wtsb: (l*2+0)=E (k=3),
    # (l*2+1)=B (k=1)
    wtsf = np.concatenate(
        [np.stack([lin_w[l, 2], lin_w[l, 0]]) for l in range(L)]
    ).reshape(L * 2 * D, D).astype(np.float32)
    wtsb = np.concatenate(
        [np.stack([lin_w[l, 3], lin_w[l, 1]]) for l in range(L)]
    ).reshape(L * 2 * D, D).astype(ml_dtypes.bfloat16)

    bias_eb = np.zeros((P, L * 2 * D), np.float32)
    bias_da = np.zeros((P, L * 2 * D), np.float32)
    for l in range(L):
        bias_eb[:, l * 2 * D:l * 2 * D + D] = lin_b[l, 3][None, :]
        bias_eb[:, l * 2 * D + D:(l + 1) * 2 * D] = lin_b[l, 1][None, :]
        bias_da[:, l * 2 * D:l * 2 * D + D] = lin_b[l, 2][None, :]
        bias_da[:, l * 2 * D + D:(l + 1) * 2 * D] = lin_b[l, 0][None, :]
    gbv = np.zeros((1, L * 2 * D), np.float32)
    for l in range(L):
        gbv[0, l * 2 * D:l * 2 * D + D] = np.asarray(bn_gamma, np.float32)[l]
        gbv[0, l * 2 * D + D:(l + 1) * 2 * D] = np.asarray(bn_beta,
                                                           np.float32)[l]

    node_ids = np.arange(NPAD)
    maskv = (node_ids < N_REAL).astype(np.float32)
    bslotv = np.full(NPAD, -1.0, np.float32)
    bslotv[:N_REAL] = bv.astype(np.float32)

    shared = dict(
        xf0=xf0, wtsf=wtsf, wtsb=wtsb, bias_eb=bias_eb, bias_da=bias_da,
        gb=gbv,
        iota128=np.tile(np.arange(P, dtype=ml_dtypes.bfloat16)[None, :],
                        (P, 1)),
        iota64=np.tile(np.arange(G, dtype=np.float32)[None, :], (P, 1)),
        ident=np.eye(P, dtype=np.float32),
        ones1=np.ones((1, P), np.float32),
        onescol=np.ones((P, 1), np.float32),
        pw1=np.asarray(post_w1, np.float32),
        pb1b=np.tile(np.asarray(post_b1, np.float32)[None, :], (G, 1)),
        pw2=np.asarray(post_w2, np.float32),
        rcnt=(1.0 / np.maximum(
            np.bincount(bv, minlength=G), 1.0)).astype(
                np.float32).reshape(G, 1),
        pb2b=np.tile(np.asarray(post_b2, np.float32)[None, :], (G, 1)),
    )
    in_maps = []
    for c in range(NCORES):
        lo = c * SH
        m = dict(shared)
        m["xn0"] = np.ascontiguousarray(x0[lo:lo + SH])
        m["pk"] = np.ascontiguousarray(pk[c])
        m["sct"] = np.ascontiguousarray(sc[c])
        m["indt"] = np.ascontiguousarray(indt[c])
        m["maskT"] = np.ascontiguousarray(maskv[lo:lo + SH].reshape(WPC, P).T)
        m["bslotT"] = np.ascontiguousarray(
            bslotv[lo:lo + SH].reshape(WPC, P).T)
        in_maps.append(m)
    return in_maps, tws


def kernel(x, edge_attr, edge_index, batch_vec, lin_w, lin_b, bn_gamma,
           bn_beta, post_w1, post_b1, post_w2, post_b2, **_unused):
    in_maps, tws = _prepare_inputs(
        x, edge_index, batch_vec, lin_w, lin_b, bn_gamma, bn_beta,
        post_w1, post_b1, post_w2, post_b2)
    key = tuple(tws)
    if key not in _cache:
        _cache[key] = _build(tws)
    nc = _cache[key]
    res = run_bass_kernel_spmd(nc, in_maps, core_ids=list(range(NCORES)))
    return np.asarray(res.results[0]["out"], np.float32)

